# revision 21
# baseline (speedup 1.0000x reference)
"""F1-score (histogram_binning) Trainium2 Bass kernel — rowmax formulation.

The reference F1 epilogue only consumes diag(cm), cm[:,0], cm[:,1],
cm[0,:], cm[1,:] — not the full confusion matrix. Those five vectors
derive from three per-sample booleans plus tiny label bincounts:

  match[s] = (y_pred[s, y_true[s]] >= rowmax[s])   <=>  pred == true
  p0[s]    = (y_pred[s, 0] >= rowmax[s])           <=>  pred == 0
  p1[s]    = (y_pred[s, 1] >= rowmax[s]) & ~p0[s]  <=>  pred == 1
  (argmax is first-max, so the >= comparisons are exact)

The only non-trivial device quantity is rowmax — everything else is O(N)
scalar work. So the kernel is the pure memory-roofline loop.

Key HW findings driving the structure (from NTFF profiles):
  - the DGE delivers the FINAL completion-semaphore increment of a DMA
    instruction only after it has pushed the next same-queue
    instruction's descriptors, so the consumer-visible completion lags
    the data by ~(instr bytes)/(per-queue bandwidth). Small 512 KiB DMA
    instructions (G=8 -> 64 descriptors of 8 KiB) keep that lag ~2.5us
    and the pipeline never locks to the laggy control plane. A strict
    1:1 DMA:reduce pairing keeps the ring-semaphore rotation free
    (multi-writer tiles stall the 4-sem/ring recycling).
  - the LAST instruction on each ring has no successor to flush its
    completion marker (observed +5-8us); the tail therefore ends with a
    run of small sub-block loads whose rowmax columns are stored inline
    so every data-carrying instruction has a flushing successor. The
    ~6us post-stream tail is VectorE clearing its marker-lag-bound
    backlog (~4 blocks); finer tapering cannot shrink it because
    per-instruction overhead (~100ns) exceeds the 0.11us/block slack
    (half-blocks ~break even, quarter-blocks run a deficit).
  - fp32 tensor_reduce is 1 elem/cycle (1x uop only): 1024-elem reduces
    (~1.17us) sit just under the 512 KiB delivery period at ~410 GB/s.
  - 410 GB/s/core IS the engine ceiling for 4 KiB descriptors: midstream
    the 16 DMA engines run gap-free at ~25.3 B/ns each. 8 KiB
    descriptors (G=16) add ~2% engine rate but lose overall: with one
    reduce per block the rotation feedback collapses the pipeline to
    ~330 GB/s; with two half-reduces per block it is stable but the
    doubled completion-marker lag still lands ~12us slower (193us vs
    181us measured). Occasional ~210-255us draws
    are environmental (chip HBM arbitration across the 8 cores); buffer
    depth 12 vs 24 vs 40 neither causes nor prevents them.

Host: gather x_true, 3M float compares for the masks, five bincounts,
argmax of the ~16k rows with true<=1 (cm rows 0/1), exact fp32 epilogue.
Bit-exact vs the jax reference (validated: rel err 0.0).
"""

import sys

import numpy as np

sys.path.insert(0, "/opt/trn_rl_repo")

import concourse.bacc as bacc  # noqa: E402
import concourse.tile as tile  # noqa: E402
from concourse import mybir  # noqa: E402
from concourse.bass_utils import run_bass_kernel_spmd  # noqa: E402

N_CORES = 8
N_SAMPLES = 1048576
C = 128
EPS = 1e-07
N_PER_CORE = N_SAMPLES // N_CORES  # 131072
P = 128  # partitions
F_PER_PART = N_PER_CORE // P  # 1024 samples per partition
G = 8  # samples per partition per block (512 KiB DMA instructions)
N_BLOCKS = F_PER_PART // G  # 128 blocks
CHUNK = 32  # blocks per rowmax store chunk
N_TAIL = 2  # final blocks issued as half-block sub-DMAs


def build_program(bufs=12):
    nc = bacc.Bacc("TRN2")

    y_pred = nc.dram_tensor(
        "y_pred", [N_PER_CORE, C], mybir.dt.float32, kind="ExternalInput"
    )
    rowmax_t = nc.dram_tensor(
        "rowmax", [P, F_PER_PART], mybir.dt.float32, kind="ExternalOutput"
    )

    # sample s_local = p * F_PER_PART + b*G + g -> contiguous per-partition DMA
    xs = y_pred[:].rearrange("(p b g) c -> p b g c", p=P, b=N_BLOCKS, g=G)

    with tile.TileContext(nc) as tc:
        with (
            tc.tile_pool(name="consts", bufs=1) as consts,
            tc.tile_pool(name="xp", bufs=bufs) as xp,
        ):
            rm_all = consts.tile([P, F_PER_PART], mybir.dt.float32, tag="rm")

            for b in range(N_BLOCKS):
                x_t = xp.tile([P, G, C], mybir.dt.float32)
                dma_eng = nc.sync if b % 2 == 0 else nc.scalar
                if b < N_BLOCKS - N_TAIL:
                    dma_eng.dma_start(out=x_t, in_=xs[:, b])
                    nc.vector.tensor_reduce(
                        out=rm_all[:, b * G : (b + 1) * G],
                        in_=x_t,
                        axis=mybir.AxisListType.X,
                        op=mybir.AluOpType.max,
                    )
                else:
                    # tail: half-block sub-DMAs so each instruction's
                    # completion marker is flushed by the next one's
                    # descriptor push within ~0.6us of its data
                    for j in range(2):
                        gj = slice(j * (G // 2), (j + 1) * (G // 2))
                        i = 2 * (b - (N_BLOCKS - N_TAIL)) + j
                        sub_eng = nc.sync if i % 2 == 0 else nc.scalar
                        sub_eng.dma_start(out=x_t[:, gj], in_=xs[:, b, gj])
                        nc.vector.tensor_reduce(
                            out=rm_all[:, b * G + j * (G // 2) :][:, : G // 2],
                            in_=x_t[:, gj],
                            axis=mybir.AxisListType.X,
                            op=mybir.AluOpType.max,
                        )
                    # store this tail block's rowmax columns immediately (32
                    # B/partition) -- waits only on the reduces issued so far,
                    # and each store's marker is flushed by the other ring's
                    # remaining tail instructions
                    st_eng = nc.scalar if b == N_BLOCKS - 2 else nc.sync
                    st_eng.dma_start(
                        out=rowmax_t[:, b * G : (b + 1) * G],
                        in_=rm_all[:, b * G : (b + 1) * G],
                    )
                if b % CHUNK == CHUNK - 1 and b < N_BLOCKS - CHUNK:
                    k = b // CHUNK
                    ck = slice(k * CHUNK * G, (k + 1) * CHUNK * G)
                    nc.gpsimd.dma_start(out=rowmax_t[:, ck], in_=rm_all[:, ck])
                elif b == N_BLOCKS - N_TAIL - 1:
                    # last chunk minus the tail blocks: store while the
                    # stream is still running
                    ck = slice((N_BLOCKS - CHUNK) * G, (N_BLOCKS - N_TAIL) * G)
                    nc.gpsimd.dma_start(out=rowmax_t[:, ck], in_=rm_all[:, ck])

    nc.finalize()
    return nc


_PROGRAMS = {}


def _get_program(bufs=12):
    if bufs not in _PROGRAMS:
        _PROGRAMS[bufs] = build_program(bufs)
    return _PROGRAMS[bufs]


def _shard_inputs(y_pred):
    in_maps = []
    for c in range(N_CORES):
        sl = slice(c * N_PER_CORE, (c + 1) * N_PER_CORE)
        in_maps.append({"y_pred": y_pred[sl]})
    return in_maps


def _assemble(y_pred, y_true, rowmax):
    """Exact F1 from rowmax + tiny host bincounts (validated vs reference)."""
    y_true = np.asarray(y_true).astype(np.int64)
    x_true = np.take_along_axis(y_pred, y_true[:, None], axis=1)[:, 0]
    match = x_true >= rowmax
    p0 = y_pred[:, 0] >= rowmax
    p1 = (y_pred[:, 1] >= rowmax) & ~p0  # exact pred==1 even under 0-1 ties

    TP = np.bincount(y_true[match], minlength=C).astype(np.float32)
    col0 = np.bincount(y_true[p0], minlength=C).astype(np.float32)
    col1 = np.bincount(y_true[p1], minlength=C).astype(np.float32)
    sel = y_true <= 1
    pred_sel = np.argmax(y_pred[sel], axis=1)
    t_sel = y_true[sel]
    row0 = np.bincount(pred_sel[t_sel == 0], minlength=C).astype(np.float32)
    row1 = np.bincount(pred_sel[t_sel == 1], minlength=C).astype(np.float32)

    FP = np.float32(C - 1) * col1 + col0
    FN = np.float32(C - 1) * row1 + row0
    eps = np.float32(EPS)
    sensitivity = np.mean(TP / (TP + FN + eps), dtype=np.float32)
    precision = np.mean(TP / (TP + FP + eps), dtype=np.float32)
    f1 = np.float32(2.0) * (precision * sensitivity / (precision + sensitivity + eps))
    return np.asarray(f1, dtype=np.float32)


def run_on_device(y_pred, y_true, bufs=12, **kwargs):
    """Run the bass kernel on 8 cores; returns (rowmax[N], results_obj)."""
    nc = _get_program(bufs)
    in_maps = _shard_inputs(y_pred)
    res = run_bass_kernel_spmd(nc, in_maps, core_ids=list(range(N_CORES)), **kwargs)
    # rowmax[p, t] covers sample s_local = p*1024 + t -> flat concat is exact
    rowmax = np.concatenate(
        [np.asarray(r["rowmax"], dtype=np.float32).reshape(-1) for r in res.results]
    )
    return rowmax, res


def kernel(y_pred, y_true):
    y_pred = np.ascontiguousarray(np.asarray(y_pred), dtype=np.float32)
    rowmax, _ = run_on_device(y_pred, y_true)
    return _assemble(y_pred, y_true, rowmax)
